# revision 1
# baseline (speedup 1.0000x reference)
"""CharRNN Trainium2 kernel.

Math (reference):
    x_embed = embedding[x]                      # [B, L, E]
    h_{t+1} = tanh([x_t, h_t] @ W_h + b_h)      # scan over L
    logits  = outs @ W_o + b_o                  # [B, L, V]

Device strategy (8 NeuronCores, no collectives):
  - Split W_h into W_e = W_h[:E] and W_hh = W_h[E:].
  - Phase A: EMB_PROJ[v, :] = embedding[v] @ W_e + b_h   ([V, H], bf16, DRAM)
    so the per-step input projection becomes a row gather: pre_t = EMB_PROJ[x_t].
  - Phase B (replicated on all cores): recurrence in transposed layout
    hT[d, b] with d-on-partitions.  Per step, for each output chunk j:
      psum[:, j] = pre_t[:, j-chunk].T          (matmul lhsT=pre chunk, rhs=I_32)
      psum[:, j] += sum_k W_hh[k-chunk, j-chunk].T-contracted with hT[k-chunk]
    then hT_next = tanh(psum) on ScalarE — output lands directly in hT layout.
    hT is also streamed to DRAM as OUTS^T [H, B*L] for phase C.
  - Phase C (vocab-sharded): logits^T[v, tb] = sum_k W_o[k, v-shard] x OUTS^T,
    W_o tiles stationary, OUTS^T streaming, + b_o via ScalarE per-partition bias.
  - Host: gathers/transposes/concats the per-core logits^T into [B, L, V].
"""

import os
import sys
import time

sys.path.insert(0, "/opt/trn_rl_repo")

import numpy as np
import ml_dtypes

from concourse import bacc, bass, mybir
import concourse.tile as tile
from concourse.bass_utils import run_bass_kernel_spmd

B, L, H, E, V = 32, 1024, 1024, 512, 8192
N_CORES = 8
VS = V // N_CORES  # vocab shard per core
BF16 = ml_dtypes.bfloat16
dt = mybir.dt

KP = H // 128  # 8 k-chunks of hidden dim
JP = H // 128  # 8 output chunks of hidden dim


def build_program(steps=L):
    BL = B * steps
    nc = bacc.Bacc("TRN2", target_bir_lowering=False, debug=False,
                   num_devices=N_CORES)

    embT = nc.dram_tensor("embT", [E, V], dt.bfloat16, kind="ExternalInput").ap()
    we = nc.dram_tensor("we", [E, H], dt.bfloat16, kind="ExternalInput").ap()
    whh = nc.dram_tensor("whh", [H, H], dt.bfloat16, kind="ExternalInput").ap()
    bh_bc = nc.dram_tensor("bh_bc", [128, H], dt.float32, kind="ExternalInput").ap()
    xin = nc.dram_tensor("x", [B, steps], dt.int32, kind="ExternalInput").ap()
    h0T = nc.dram_tensor("h0T", [H, B], dt.float32, kind="ExternalInput").ap()
    wo = nc.dram_tensor("wo", [H, VS], dt.bfloat16, kind="ExternalInput").ap()
    bo = nc.dram_tensor("bo", [128, VS // 128], dt.float32, kind="ExternalInput").ap()
    ident_in = nc.dram_tensor("ident", [B, B], dt.bfloat16, kind="ExternalInput").ap()

    logitsT = nc.dram_tensor("logitsT", [VS, BL], dt.float32,
                             kind="ExternalOutput").ap()
    fhT = nc.dram_tensor("fhT", [H, B], dt.float32, kind="ExternalOutput").ap()

    with tile.TileContext(nc) as tc:
        with tc.tile_pool(name="dram", bufs=1, space="DRAM") as dpool, \
             tc.tile_pool(name="persist", bufs=1) as pp:
            eproj = dpool.tile([V, H], dt.bfloat16)
            outsT = dpool.tile([H, BL], dt.bfloat16)

            # persistent SBUF: weights + indices + identity
            whh_sb = pp.tile([128, KP * H], dt.bfloat16)  # k-chunk k at cols [H*k : H*(k+1)]
            for k in range(KP):
                nc.sync.dma_start(out=whh_sb[:, H * k:H * (k + 1)],
                                  in_=whh[128 * k:128 * (k + 1), :])
            wo_sb = pp.tile([128, KP * VS], dt.bfloat16)
            for k in range(KP):
                nc.sync.dma_start(out=wo_sb[:, VS * k:VS * (k + 1)],
                                  in_=wo[128 * k:128 * (k + 1), :])
            x_sb = pp.tile([B, steps], dt.int32)
            nc.sync.dma_start(out=x_sb[:], in_=xin[:])
            bo_sb = pp.tile([128, VS // 128], dt.float32)
            nc.sync.dma_start(out=bo_sb[:], in_=bo[:])
            ident = pp.tile([B, B], dt.bfloat16)
            nc.sync.dma_start(out=ident[:], in_=ident_in[:])

            # initial hidden, transposed layout: hT[p, 32k+b] = h[b, 128k+p]
            h0f = pp.tile([128, KP * B], dt.float32)
            nc.sync.dma_start(
                out=h0f[:].rearrange("p (k b) -> p k b", k=KP),
                in_=h0T[:].rearrange("(k p) b -> p k b", p=128))
            hT_prev = pp.tile([128, KP * B], dt.bfloat16)
            nc.vector.tensor_copy(out=hT_prev[:], in_=h0f[:])

            # ---------------- Phase A: EMB_PROJ = emb @ W_e + b_h ----------
            with tc.tile_pool(name="pa_we", bufs=1) as pa_we, \
                 tc.tile_pool(name="pa_in", bufs=3) as pa_in, \
                 tc.tile_pool(name="pa_ps", bufs=4, space="PSUM") as pa_ps, \
                 tc.tile_pool(name="pa_out", bufs=3) as pa_out:
                we_sb = pa_we.tile([128, 4 * H], dt.bfloat16)
                for ec in range(4):
                    nc.sync.dma_start(out=we_sb[:, H * ec:H * (ec + 1)],
                                      in_=we[128 * ec:128 * (ec + 1), :])
                bh_sb = pa_we.tile([128, H], dt.float32)
                nc.sync.dma_start(out=bh_sb[:], in_=bh_bc[:])

                for vc in range(V // 128):
                    embt_t = pa_in.tile([128, 4 * 128], dt.bfloat16)
                    for ec in range(4):
                        nc.sync.dma_start(
                            out=embt_t[:, 128 * ec:128 * (ec + 1)],
                            in_=embT[128 * ec:128 * (ec + 1),
                                     128 * vc:128 * (vc + 1)])
                    for nh in range(2):
                        ps = pa_ps.tile([128, 512], dt.float32, space="PSUM")
                        for ec in range(4):
                            nc.tensor.matmul(
                                out=ps[:],
                                lhsT=embt_t[:, 128 * ec:128 * (ec + 1)],
                                rhs=we_sb[:, H * ec + 512 * nh:H * ec + 512 * (nh + 1)],
                                start=(ec == 0), stop=(ec == 3))
                        ot = pa_out.tile([128, 512], dt.bfloat16)
                        nc.vector.tensor_tensor(
                            out=ot[:], in0=ps[:],
                            in1=bh_sb[:, 512 * nh:512 * (nh + 1)],
                            op=mybir.AluOpType.add)
                        nc.sync.dma_start(
                            out=eproj[128 * vc:128 * (vc + 1),
                                      512 * nh:512 * (nh + 1)],
                            in_=ot[:])

            # ---------------- Phase B: recurrence --------------------------
            with tc.tile_pool(name="pb_pre", bufs=8) as pb_pre, \
                 tc.tile_pool(name="pb_ps", bufs=2, space="PSUM") as pb_ps, \
                 tc.tile_pool(name="pb_h", bufs=3) as pb_h, \
                 tc.tile_pool(name="pb_fh", bufs=1) as pb_fh:
                for t in range(steps):
                    pre_t = pb_pre.tile([B, H], dt.bfloat16)
                    nc.gpsimd.indirect_dma_start(
                        out=pre_t[:], out_offset=None,
                        in_=eproj[:],
                        in_offset=bass.IndirectOffsetOnAxis(
                            ap=x_sb[:, t:t + 1], axis=0))
                    ps = pb_ps.tile([128, JP * B], dt.float32, space="PSUM")
                    for j in range(JP):
                        oslice = ps[:, B * j:B * (j + 1)]
                        nc.tensor.matmul(
                            out=oslice,
                            lhsT=pre_t[:, 128 * j:128 * (j + 1)],
                            rhs=ident[:], start=True, stop=False)
                        for k in range(KP):
                            nc.tensor.matmul(
                                out=oslice,
                                lhsT=whh_sb[:, H * k + 128 * j:H * k + 128 * (j + 1)],
                                rhs=hT_prev[:, B * k:B * (k + 1)],
                                start=False, stop=(k == KP - 1))
                    hT_next = pb_h.tile([128, JP * B], dt.bfloat16)
                    nc.scalar.activation(out=hT_next[:], in_=ps[:],
                                         func=mybir.ActivationFunctionType.Tanh)
                    nc.sync.dma_start(
                        out=outsT[:, B * t:B * (t + 1)].rearrange(
                            "(k p) b -> p k b", p=128),
                        in_=hT_next[:].rearrange("p (k b) -> p k b", k=KP))
                    if t == steps - 1:
                        fh_sb = pb_fh.tile([128, JP * B], dt.float32)
                        nc.scalar.activation(
                            out=fh_sb[:], in_=ps[:],
                            func=mybir.ActivationFunctionType.Tanh)
                        nc.sync.dma_start(
                            out=fhT[:].rearrange("(k p) b -> p k b", p=128),
                            in_=fh_sb[:].rearrange("p (k b) -> p k b", k=KP))
                    hT_prev = hT_next

            # ---------------- Phase C: logits^T ----------------------------
            NBLK = BL // 512
            with tc.tile_pool(name="pc_in", bufs=3) as pc_in, \
                 tc.tile_pool(name="pc_ps", bufs=4, space="PSUM") as pc_ps, \
                 tc.tile_pool(name="pc_out", bufs=3) as pc_out:
                for blk in range(NBLK):
                    ob = pc_in.tile([128, KP * 512], dt.bfloat16)
                    nc.sync.dma_start(
                        out=ob[:].rearrange("p (k n) -> p k n", k=KP),
                        in_=outsT[:, 512 * blk:512 * (blk + 1)].rearrange(
                            "(k p) n -> p k n", p=128))
                    for vc in range(VS // 128):
                        ps = pc_ps.tile([128, 512], dt.float32, space="PSUM")
                        for k in range(KP):
                            nc.tensor.matmul(
                                out=ps[:],
                                lhsT=wo_sb[:, VS * k + 128 * vc:VS * k + 128 * (vc + 1)],
                                rhs=ob[:, 512 * k:512 * (k + 1)],
                                start=(k == 0), stop=(k == KP - 1))
                        lg = pc_out.tile([128, 512], dt.float32)
                        nc.scalar.activation(
                            out=lg[:], in_=ps[:],
                            func=mybir.ActivationFunctionType.Identity,
                            bias=bo_sb[:, vc:vc + 1], scale=1.0)
                        nc.sync.dma_start(
                            out=logitsT[128 * vc:128 * (vc + 1),
                                        512 * blk:512 * (blk + 1)],
                            in_=lg[:])

    nc.compile()
    return nc


def prep_inputs(x, hidden, embedding, W_h, b_h, W_o, b_o, steps=L):
    x = np.asarray(x)
    x_i32 = np.ascontiguousarray(x[:, :steps].astype(np.int32))
    emb = np.asarray(embedding, dtype=np.float32)
    W_h = np.asarray(W_h, dtype=np.float32)
    b_h = np.asarray(b_h, dtype=np.float32)
    W_o = np.asarray(W_o, dtype=np.float32)
    b_o = np.asarray(b_o, dtype=np.float32)
    hidden = np.asarray(hidden, dtype=np.float32)

    embT_bf = np.ascontiguousarray(emb.T).astype(BF16)
    we_bf = np.ascontiguousarray(W_h[:E]).astype(BF16)
    whh_bf = np.ascontiguousarray(W_h[E:]).astype(BF16)
    bh_bc = np.ascontiguousarray(np.broadcast_to(b_h[None, :], (128, H)))
    h0T = np.ascontiguousarray(hidden.T)
    ident = np.eye(B, dtype=BF16)

    common = dict(embT=embT_bf, we=we_bf, whh=whh_bf, bh_bc=bh_bc,
                  x=x_i32, h0T=h0T, ident=ident)
    in_maps = []
    for c in range(N_CORES):
        wo_c = np.ascontiguousarray(W_o[:, c * VS:(c + 1) * VS]).astype(BF16)
        bo_c = np.ascontiguousarray(
            b_o[c * VS:(c + 1) * VS].reshape(VS // 128, 128).T)
        in_maps.append(dict(common, wo=wo_c, bo=bo_c))
    return in_maps


def assemble_outputs(results, steps=L):
    # logitsT per core: [VS, B*steps] with column index = 32*t + b
    logits = np.empty((B, steps, V), dtype=np.float32)
    for c in range(N_CORES):
        lt = results[c]["logitsT"]  # [VS, B*steps]
        # -> [steps, B, VS] -> [B, steps, VS]
        logits[:, :, c * VS:(c + 1) * VS] = (
            lt.T.reshape(steps, B, VS).transpose(1, 0, 2))
    final_hidden = np.ascontiguousarray(results[0]["fhT"].T)
    return logits, final_hidden


_PROGRAM_CACHE = {}


def run(inputs_dict, steps=L, **spmd_kwargs):
    nc = _PROGRAM_CACHE.get(steps)
    if nc is None:
        nc = build_program(steps)
        _PROGRAM_CACHE[steps] = nc
    in_maps = prep_inputs(steps=steps, **inputs_dict)
    res = run_bass_kernel_spmd(nc, in_maps, list(range(N_CORES)), **spmd_kwargs)
    return assemble_outputs(res.results, steps=steps)


def kernel(x, hidden, embedding, W_h, b_h, W_o, b_o):
    logits, final_hidden = run(
        dict(x=x, hidden=hidden, embedding=embedding, W_h=W_h,
             b_h=b_h, W_o=W_o, b_o=b_o))
    return logits, final_hidden


if __name__ == "__main__":
    # tiny smoke run with random data at reduced length
    steps = int(os.environ.get("STEPS", "64"))
    rng = np.random.default_rng(0)
    inputs = dict(
        x=rng.integers(0, V, size=(B, L)).astype(np.int32),
        hidden=np.zeros((B, H), np.float32),
        embedding=rng.standard_normal((V, E), dtype=np.float32),
        W_h=(rng.standard_normal((E + H, H), dtype=np.float32) * 0.01),
        b_h=np.zeros((H,), np.float32),
        W_o=(rng.standard_normal((H, V), dtype=np.float32) * 0.01),
        b_o=np.zeros((V,), np.float32),
    )
    t0 = time.time()
    logits, fh = run(inputs, steps=steps)
    print("run wall:", time.time() - t0)

    # numpy reference at same steps
    xs = inputs["x"][:, :steps]
    emb = inputs["embedding"]
    W_h_, b_h_, W_o_, b_o_ = (inputs["W_h"], inputs["b_h"],
                              inputs["W_o"], inputs["b_o"])
    h = inputs["hidden"].copy()
    outs = np.zeros((B, steps, H), np.float32)
    for t in range(steps):
        xt = emb[xs[:, t]]
        h = np.tanh(np.concatenate([xt, h], axis=1) @ W_h_ + b_h_)
        outs[:, t] = h
    ref_logits = outs @ W_o_ + b_o_
    err = np.linalg.norm(logits - ref_logits) / np.linalg.norm(ref_logits)
    errh = np.linalg.norm(fh - h) / np.linalg.norm(h)
    print("rel err logits:", err, " rel err fh:", errh)
    print("max abs err:", np.abs(logits - ref_logits).max())
